# revision 39
# baseline (speedup 1.0000x reference)
"""Trainium2 Bass kernel for the GNN message-update MLP:

    out = relu(concat([v_i, v_j, e_ij], -1) @ W1 + b1) @ W2 + b2

Strategy (memory-bound, E = 1M edges, data-parallel across 8 cores):
  - Shard edges across the 8 NeuronCores (125000 each; 30 full 4096-edge
    blocks + one 3072-edge tail block).
  - Pure fp16 I/O: activations ship as fp16 (half the HBM bytes of fp32)
    and the output is written back as fp16, converted to fp32 on host.
    PSUM accumulation stays fp32; end-to-end error ~6e-4 of scale
    (harness gate is 2e-2; e_ij in fp8-e4m3 measured 2.0e-2 - rejected).
  - Per 1024-edge pair (two 512-edge tiles on PSUM row halves via column
    tile_position): 2x K=128 x-matmuls (these co-execute on disjoint PE
    column groups), ONE K=64 full-width e-matmul with blockdiag(We, We)
    against the pair's partition-stacked e rows, and ONE full-width
    layer-2 matmul with blockdiag(W2, W2). 4 matmuls / 1024 edges.
  - One [128,512] VectorE relu+bias (fp32 PSUM -> fp16) and one
    [128,512] ScalarE copy (PSUM -> fp16 SBUF) per pair - all
    element-wise work runs on full 128 partitions.
  - Layer-2 + output copy are software-pipelined three pairs behind
    layer-1 so the PE queue never stalls on the vector engine or DMA
    jitter; any >~1.3us PE gap drops the HAM clock gate to 1.2 GHz and
    a saturated cold PE rarely re-raises (hence also the 12-matmul
    warmup block and the chunked first-block DMA for a gap-free
    warmup -> real-work handoff).
  - Inputs stream on the sync-engine HWDGE queue, outputs on the
    scalar-engine HWDGE queue; the two concurrent queues together run
    the HBM interface at its practical limit (~400 GB/s aggregate).
"""

import numpy as np

import concourse.bacc as bacc
import concourse.mybir as mybir
import concourse.tile as tile
from concourse.bass_utils import run_bass_kernel_spmd

# ---- problem constants (hardcoded per harness contract) ----
E_TOTAL = 1_000_000
N_CORES = 8
IN_C = 64
IN_E = 32
HID = 64
OUT_C = 64

NHALF = 512                    # edges per 64-col output tile / matmul N
Q_PER_BLK = 8                  # 512-edge tiles per block
P_PER_BLK = Q_PER_BLK // 2     # 4 pairs per block
BLK_EDGES = NHALF * Q_PER_BLK  # 4096
EPC = E_TOTAL // N_CORES       # 125000 edges per core
N_BLK = -(-EPC // BLK_EDGES)   # 31
EPAD = N_BLK * BLK_EDGES       # 126976
# pairs per block: full blocks have 4; the tail block only covers the
# 2120 leftover edges -> 3 pairs (3072 edges), trimming pad DMA+compute
P_LAST = -(-(EPC - (N_BLK - 1) * BLK_EDGES) // (2 * NHALF))  # 3

ECOLS = BLK_EDGES // 4         # 1024 e-columns per block (32-row bands)
XBASE = ECOLS                  # x-columns start after the e-columns
INCOLS = BLK_EDGES + ECOLS     # 5120

F32 = mybir.dt.float32
F16 = mybir.dt.float16

# test.py hooks
_TRACE = False
LAST_RESULT = None

_PROGRAM_CACHE = {}


def _build_program():
    nc = bacc.Bacc(
        "TRN2",
        target_bir_lowering=False,
        debug=False,
        num_devices=N_CORES,
    )

    xin = nc.declare_dram_parameter(
        "xin", [N_BLK, 128, INCOLS], F16, isOutput=False
    )
    wx = nc.declare_dram_parameter("wx", [128, HID], F16, isOutput=False)
    wes2d = nc.declare_dram_parameter("wes2d", [128, 128], F16, isOutput=False)
    w2d = nc.declare_dram_parameter("w2d", [128, 128], F16, isOutput=False)
    b1r = nc.declare_dram_parameter("b1r", [128, 1], F32, isOutput=False)
    out = nc.declare_dram_parameter(
        "out", [N_BLK, 128, P_PER_BLK * NHALF], F16, isOutput=True
    )

    with tile.TileContext(nc) as tc:
        with (
            tc.tile_pool(name="consts", bufs=1) as cpool,
            tc.tile_pool(name="xi", bufs=4) as xi_pool,
            tc.tile_pool(name="hh", bufs=5) as hh_pool,
            tc.tile_pool(name="ob", bufs=3) as ob_pool,
            tc.tile_pool(name="ph", bufs=4, space="PSUM") as ph_pool,
            tc.tile_pool(name="po", bufs=4, space="PSUM") as po_pool,
        ):
            wx_t = cpool.tile([128, HID], F16)
            wes2d_t = cpool.tile([128, 128], F16)
            w2d_t = cpool.tile([128, 128], F16)
            b1r_t = cpool.tile([128, 1], F32)

            # Warm the PE clock gate (HAM): a dense block of full-array
            # matmuls reliably raises the PE clock 1.2 -> 2.4 GHz ~7us in
            # (the quadrant-tiled real stream alone never triggers the
            # raise, even when gap-free). The raised clock then sticks as
            # long as the real stream avoids >~1.3us PE stalls.
            warm_t = cpool.tile([128, NHALF], F16)
            nc.vector.memset(warm_t[:], 0.0)
            warm_ps = ph_pool.tile([128, NHALF], F32, tag="ph_t", name="warm_ps")
            for _ in range(12):
                nc.tensor.matmul(
                    warm_ps[:, :], warm_t[:, 0:128], warm_t[:, :],
                    start=True, stop=True,
                )

            # software pipeline: layer-2 runs THREE pairs behind layer-1
            # so the PE queue neither waits on the vector engine's
            # relu+sem latency nor on input-DMA jitter.
            # entries: (hh tile, ob tile, pair idx, blk)
            pending = []

            def emit_l2(p):
                hh, ob_t, pr, b, npr = p
                po = po_pool.tile([128, NHALF], F32, tag="po_t", name="po")
                nc.tensor.matmul(
                    po[:, :], w2d_t[:, :], hh[:, :],
                    start=True, stop=True, tile_position=(0, 0),
                )
                nc.scalar.activation(
                    ob_t[:, pr * NHALF : (pr + 1) * NHALF], po[:, :],
                    mybir.ActivationFunctionType.Copy,
                )
                if pr == npr - 1:
                    nc.scalar.dma_start(
                        out[b, :, 0 : npr * NHALF], ob_t[:, 0 : npr * NHALF]
                    )

            for blk in range(N_BLK):
                xi_t = xi_pool.tile([128, INCOLS], F16)
                # Early blocks: chunked input DMA (e-columns first) so
                # completion semaphores pace ahead of PE consumption and
                # the warmup->real handoff has no PE idle gap. Steady
                # state: one big DMA per block (fewer packets -> better
                # HBM efficiency); the 4-5 block lookahead hides the
                # completion latency.
                if blk == 0:
                    # block-0 chunks + weights issue from the SCALAR
                    # engine's HWDGE ring: its preamble is ~1.3us vs the
                    # sync engine's ~8us (DRAIN + table loads), so the
                    # first chunk semaphores land BEFORE warmup ends and
                    # the PE never idles at the warmup->real handoff.
                    # The ring is otherwise unused until the first
                    # output DMA (~24us).
                    for ck in range(2):
                        c0 = ck * ECOLS
                        nc.scalar.dma_start(
                            xi_t[:, c0 : c0 + ECOLS],
                            xin[blk, :, c0 : c0 + ECOLS],
                        )
                    nc.scalar.dma_start(wx_t[:], wx[:])
                    nc.scalar.dma_start(wes2d_t[:], wes2d[:])
                    nc.scalar.dma_start(w2d_t[:], w2d[:])
                    nc.scalar.dma_start(b1r_t[:], b1r[:])
                    for ck in range(2, 5):
                        c0 = ck * ECOLS
                        nc.scalar.dma_start(
                            xi_t[:, c0 : c0 + ECOLS],
                            xin[blk, :, c0 : c0 + ECOLS],
                        )
                elif blk <= 2:
                    nc.sync.dma_start(xi_t[:, 0:ECOLS], xin[blk, :, 0:ECOLS])
                    half = (INCOLS - ECOLS) // 2
                    nc.sync.dma_start(
                        xi_t[:, ECOLS : ECOLS + half],
                        xin[blk, :, ECOLS : ECOLS + half],
                    )
                    nc.sync.dma_start(
                        xi_t[:, ECOLS + half : INCOLS],
                        xin[blk, :, ECOLS + half : INCOLS],
                    )
                else:
                    npr = P_LAST if blk == N_BLK - 1 else P_PER_BLK
                    ncols = ECOLS + npr * 2 * NHALF
                    nc.sync.dma_start(xi_t[:, 0:ncols], xin[blk, :, 0:ncols])
                ob_t = ob_pool.tile([128, P_PER_BLK * NHALF], F16)

                n_pairs = P_LAST if blk == N_BLK - 1 else P_PER_BLK
                for pr in range(n_pairs):
                    # tiles qa = 2*pr, qb = 2*pr+1 -> PSUM rows 0:64 /
                    # 64:128; both e-tiles sit stacked in one 64-row band
                    # (rows 64*(pr%2)..+64, cols 512*(pr//2)), so ONE
                    # K=64 full-width matmul with blockdiag(We, We) adds
                    # both e contributions.
                    qa, qb = 2 * pr, 2 * pr + 1
                    er = 64 * (pr % 2)
                    ec = NHALF * (pr // 2)
                    ph = ph_pool.tile([128, NHALF], F32, tag="ph_t", name="ph")
                    nc.tensor.matmul(
                        ph[0:64, :], wx_t[:, :],
                        xi_t[:, XBASE + qa * NHALF : XBASE + (qa + 1) * NHALF],
                        start=True, stop=False, tile_position=(0, 0),
                    )
                    nc.tensor.matmul(
                        ph[64:128, :], wx_t[:, :],
                        xi_t[:, XBASE + qb * NHALF : XBASE + (qb + 1) * NHALF],
                        start=True, stop=False, tile_position=(0, 64),
                    )
                    nc.tensor.matmul(
                        ph[:, :],
                        wes2d_t[er : er + 64, :],
                        xi_t[er : er + 64, ec : ec + NHALF],
                        start=False, stop=True, tile_position=(er, 0),
                        skip_group_check=True,
                    )
                    # relu(ph + b1) -> fp16, full 128 partitions
                    hh = hh_pool.tile([128, NHALF], F16, tag="hh", name="hh")
                    nc.vector.tensor_scalar(
                        hh[:, :], ph[:, :], b1r_t[:, :], 0.0,
                        mybir.AluOpType.add, mybir.AluOpType.max,
                    )
                    # layer 2 from three pairs ago (software pipelining)
                    if len(pending) == 3:
                        emit_l2(pending.pop(0))
                    pending.append((hh, ob_t, pr, blk, n_pairs))

            for p in pending:
                emit_l2(p)

    nc.compile()
    return nc


def _get_program():
    if "prog" not in _PROGRAM_CACHE:
        _PROGRAM_CACHE["prog"] = _build_program()
    return _PROGRAM_CACHE["prog"]


def _pad_rows(a, n):
    if a.shape[0] == n:
        return a
    pad = np.zeros((n - a.shape[0],) + a.shape[1:], dtype=a.dtype)
    return np.concatenate([a, pad], axis=0)


def _host_pack(v_i, v_j, e_ij, W1, b1, W2, b2):
    """Build per-core input maps in the device layouts."""
    W1 = np.asarray(W1, dtype=np.float32)
    W2 = np.asarray(W2, dtype=np.float32)
    wx_h = W1[:128].astype(np.float16)
    wes_h = W1[128:160].astype(np.float16)
    w2_h = W2.astype(np.float16)

    w2d = np.zeros((128, 128), dtype=np.float16)
    w2d[0:64, 0:64] = w2_h
    w2d[64:128, 64:128] = w2_h

    # blockdiag(We, We) [64, 128], tiled twice down the partitions so the
    # e-matmul's stationary operand sits at the same base partition as its
    # moving band (rows 0:64 or 64:128).
    wes2d_half = np.zeros((64, 128), dtype=np.float16)
    wes2d_half[0:32, 0:64] = wes_h
    wes2d_half[32:64, 64:128] = wes_h
    wes2d = np.tile(wes2d_half, (2, 1))

    weights = {
        "wx": np.ascontiguousarray(wx_h),
        "wes2d": np.ascontiguousarray(wes2d),
        "w2d": w2d,
        "b1r": np.ascontiguousarray(np.tile(b1, 2)[:, None], dtype=np.float32),
    }

    in_maps = []
    for c in range(N_CORES):
        sl = slice(c * EPC, (c + 1) * EPC)
        vi = _pad_rows(np.asarray(v_i[sl], dtype=np.float16), EPAD)
        vj = _pad_rows(np.asarray(v_j[sl], dtype=np.float16), EPAD)
        ec = _pad_rows(np.asarray(e_ij[sl], dtype=np.float16), EPAD)

        # x-part: [vi^T; vj^T] -> [N_BLK, 128, 4096]
        X = np.concatenate([vi.T, vj.T], axis=0)          # [128, EPAD] f16
        xa = X.reshape(128, N_BLK, BLK_EDGES).transpose(1, 0, 2)

        # e-part: tile q = 4h + i -> rows 32i:32i+32, cols 512h:512h+512
        ET = ec.T                                          # [32, EPAD] f16
        ebd = ET.reshape(32, N_BLK, 2, 4, NHALF).transpose(1, 3, 0, 2, 4)
        ebd = ebd.reshape(N_BLK, 128, ECOLS)               # [blk, 32i+r, 512h+n]

        xi_full = np.concatenate([ebd, xa], axis=2)        # [N_BLK, 128, 5120]
        in_maps.append({"xin": np.ascontiguousarray(xi_full), **weights})
    return in_maps


def _host_unpack(results, b2):
    """results: per-core dicts with 'out' [N_BLK, 128, 2048] f16."""
    b2 = np.asarray(b2, dtype=np.float32)
    outs = []
    for c in range(N_CORES):
        o = np.asarray(results[c]["out"])
        # o[blk, 64r + j, 512p + n] = OUT[blk*4096 + (2p + r)*512 + n, j]
        r = o.reshape(N_BLK, 2, 64, P_PER_BLK, NHALF)  # [blk, r, j, p, n]
        r = r.transpose(0, 3, 1, 4, 2)                  # [blk, p, r, n, j]
        r = np.ascontiguousarray(r).reshape(EPAD, OUT_C)[:EPC]
        outs.append(r.astype(np.float32) + b2)
    return np.concatenate(outs, axis=0)


def kernel(v_i, v_j, e_ij, W1, b1, W2, b2):
    global LAST_RESULT
    nc = _get_program()
    in_maps = _host_pack(v_i, v_j, e_ij, W1, b1, W2, b2)
    res = run_bass_kernel_spmd(
        nc, in_maps, core_ids=list(range(N_CORES)), trace=_TRACE
    )
    LAST_RESULT = res
    return _host_unpack(res.results, b2)
